# revision 14
# baseline (speedup 1.0000x reference)
"""MultiHeadSelfAttention2D Trainium2 kernel (8-core SPMD).

Sharding: core c -> (batch b = c//4, head h = c%4).
Each core: QKV 1x1-conv projections + PReLU + channel-LN for its head,
full attention over T, then an AllToAll exchanges per-head attention
outputs so each core computes the final concat projection + PReLU + LN
+ residual for a T/8 time-shard of both batches.

Attention is computed transposed: S^T[s, tq] = K^T(s-tile) . Q, exp ->
P^T tiles usable directly as matmul weights for O = P^T^T V (no PE
transposes). Softmax denominators come from a ones-column appended to V.

All shapes hardcoded for the problem instance:
  x [2, 64, 3000, 65], H=4 heads, D=4 q/k chans, E=16 v chans.
"""

import numpy as np
import ml_dtypes

import concourse.bass as bass
import concourse.mybir as mybir
import concourse.tile as tile
from concourse import bacc
from concourse.bass_utils import run_bass_kernel_spmd

BF16 = ml_dtypes.bfloat16

B, C, T, F = 2, 64, 3000, 65
H, D, E = 4, 4, 16
TP = 3072                    # padded T (24 tiles of 128)
TFP = TP * F                 # 199680 padded (t,f) positions
DF = D * F                   # 260  q/k embedding
EF = E * F                   # 1040 v embedding
SH = TP // 8                 # 384  t-shard per core per batch (final stage)
SHF = SH * F                 # 24960
SCALE = float(1.0 / np.sqrt(np.float32(DF)))
EPS = 1e-5

f32 = mybir.dt.float32
bf16 = mybir.dt.bfloat16

# projection tiling: each psum tile covers 24 consecutive t (4 col-group
# copies of 6 t each), free size 390 = 6*65; x loads batch 4 psum tiles
PJ_T = 6                  # t per copy
PJ_N = PJ_T * F           # 390 free
XSUB = 4                  # psum tiles per x load
XT = XSUB * 4 * PJ_T      # 96 t per x load
NXT = TP // XT            # 32 x loads

NQT = TP // 128           # 24 t tiles of 128
NSB = TP // 512           # 6 blocks of 512
T_PAD0 = T - 23 * 128     # 56 real rows in the last 128-tile


def _build_program(nrep=1, phases="123A5"):
    nc = bacc.Bacc("TRN2", target_bir_lowering=False, debug=False,
                   num_devices=8)

    def din(name, shape, dt=f32):
        return nc.dram_tensor(name, list(shape), dt, kind="ExternalInput")

    x_pad = din("x_pad", [C, TFP], bf16)
    x_res = din("x_res", [2 * C, SHF])
    w4 = din("w4", [C, 128], bf16)
    bias_v = din("bias_v", [120, 1])
    nbias_v = din("nbias_v", [120, 1])
    na_v = din("na_v", [120, 1])
    gam_v = din("gam_v", [120, 1])
    bet_v = din("bet_v", [120, 1])
    Gm = din("Gm", [120, 12], bf16)
    Bb = din("Bb", [12, 120], bf16)
    wp2 = din("wp2", [2 * C, 2 * C], bf16)
    ones128 = din("ones128", [2 * C, 2 * C], bf16)
    nap_v = din("nap_v", [2 * C, 1])
    bp_v = din("bp_v", [2 * C, 1])
    nbp_v = din("nbp_v", [2 * C, 1])
    gp_v = din("gp_v", [2 * C, 1])
    betp_v = din("betp_v", [2 * C, 1])

    y_out = nc.dram_tensor("y_shard", [2 * C, SHF], f32, kind="ExternalOutput")

    env = locals()
    with tile.TileContext(nc) as tc:
        for _rep in range(nrep):
            _body(tc, env, phases)
    nc.compile()
    return nc


def _body(tc, t, phases="123A5"):
    nc = tc.nc
    AP = bass.AP

    with nc.allow_low_precision(reason="bf16 staging feeds f32 psum matmuls; "
                                "LN stats tolerate bf16"), \
         tc.tile_pool(name="consts", bufs=1) as consts, \
         tc.tile_pool(name="dram", bufs=1, space="DRAM") as dram:

        # ---- constants into SBUF ----
        w4_sb = consts.tile([C, 128], bf16)
        nc.sync.dma_start(w4_sb[:], t["w4"][:])
        g_sb = consts.tile([120, 12], bf16)
        nc.sync.dma_start(g_sb[:], t["Gm"][:])
        bb_sb = consts.tile([12, 120], bf16)
        nc.sync.dma_start(bb_sb[:], t["Bb"][:])
        vecs = {}
        for nm in ("bias_v", "nbias_v", "na_v", "gam_v", "bet_v"):
            v = consts.tile([120, 1], f32, name=nm + "_sb")
            nc.sync.dma_start(v[:], t[nm][:])
            vecs[nm] = v
        fvecs = {}
        for nm in ("nap_v", "bp_v", "nbp_v", "gp_v", "betp_v"):
            v = consts.tile([2 * C, 1], f32, name=nm + "_sb")
            nc.sync.dma_start(v[:], t[nm][:])
            fvecs[nm] = v
        wp_sb = consts.tile([2 * C, 2 * C], bf16)
        nc.sync.dma_start(wp_sb[:], t["wp2"][:])
        ones_sb = consts.tile([2 * C, 2 * C], bf16)
        nc.sync.dma_start(ones_sb[:], t["ones128"][:])
        eps128 = consts.tile([128, 1], f32)
        nc.vector.memset(eps128[:], EPS)

        # ---- intermediate DRAM ----
        qkv2d = dram.tile([TP, 24 * F], bf16)   # [t, (ch, f)] ch: q0-3 k0-3 v0-15
        oint = dram.tile([8, 16 * SHF], bf16)
        oall = dram.tile([128, SHF], bf16)

        x_pad = t["x_pad"]

        # ================= phase 1: QKV proj + PReLU + LN =================
        if "1" not in phases:
            return
        with tc.tile_pool(name="p1x", bufs=2) as p1x, \
             tc.tile_pool(name="p1w", bufs=2) as p1w, \
             tc.tile_pool(name="p1s", bufs=2) as p1s, \
             tc.tile_pool(name="p1ps", bufs=2, space="PSUM") as p1ps, \
             tc.tile_pool(name="p1ps1", bufs=1, space="PSUM") as p1ps1:
            for i in range(NXT):
                t0 = i * XT
                x_tile = p1x.tile([C, XT * F], bf16, tag="x")
                nc.gpsimd.dma_start(x_tile[:], x_pad[:, t0 * F:(t0 + XT) * F])
                yf = p1w.tile([120, XSUB * PJ_N], bf16, tag="yf")

                for s in range(XSUB):
                    ypsum = p1ps.tile([128, 512], f32, tag="ypsum")
                    for j in range(4):
                        xo = (24 * j + PJ_T * s) * F
                        nc.tensor.matmul(
                            ypsum[32 * j:32 * j + 32, 0:PJ_N],
                            w4_sb[:, 32 * j:32 * j + 32],
                            x_tile[:, xo:xo + PJ_N],
                            start=True, stop=True,
                            tile_position=(0, 32 * j),
                        )
                    yp = ypsum[0:120, 0:PJ_N]

                    r1 = p1w.tile([120, PJ_N], f32, tag="r1")
                    nc.scalar.activation(r1[:], yp,
                                         mybir.ActivationFunctionType.Relu,
                                         bias=vecs["bias_v"][:], scale=1.0)
                    r2 = p1w.tile([120, PJ_N], f32, tag="r2")
                    nc.scalar.activation(r2[:], yp,
                                         mybir.ActivationFunctionType.Relu,
                                         bias=vecs["nbias_v"][:], scale=-1.0)
                    r2a = p1w.tile([120, PJ_N], f32, tag="r2a")
                    nc.vector.tensor_scalar(r2a[:], r2[:], vecs["na_v"][:],
                                            None, mybir.AluOpType.mult)
                    y_sb = p1w.tile([120, PJ_N], bf16, tag="y_sb")
                    nc.gpsimd.tensor_tensor(y_sb[:], r1[:], r2a[:],
                                            mybir.AluOpType.add)

                    mu_psf = p1ps.tile([12, 512], f32, tag="mu_ps")
                    mu_ps = mu_psf[:, 0:PJ_N]
                    nc.tensor.matmul(mu_ps, g_sb[:], y_sb[:],
                                     start=True, stop=True)
                    spair = p1s.tile([12, 2 * PJ_N], bf16, tag="spair")
                    nc.scalar.copy(spair[:, 0:PJ_N], mu_ps)
                    mub = p1ps1.tile([128, 512], f32, tag="mub")
                    nc.tensor.matmul(mub[0:120, 0:PJ_N], bb_sb[:],
                                     spair[:, 0:PJ_N], start=True, stop=True)
                    t1 = p1w.tile([120, PJ_N], f32, tag="t1")
                    nc.vector.tensor_tensor(t1[:], y_sb[:],
                                            mub[0:120, 0:PJ_N],
                                            mybir.AluOpType.subtract)
                    y2c = p1w.tile([120, PJ_N], bf16, tag="y2c")
                    nc.scalar.activation(y2c[:], t1[:],
                                         mybir.ActivationFunctionType.Square)
                    var_psf = p1ps.tile([12, 512], f32, tag="var_ps")
                    var_ps = var_psf[:, 0:PJ_N]
                    nc.tensor.matmul(var_ps, g_sb[:], y2c[:],
                                     start=True, stop=True)
                    stdd = p1s.tile([12, PJ_N], f32, tag="stdd")
                    nc.scalar.activation(stdd[:], var_ps,
                                         mybir.ActivationFunctionType.Sqrt,
                                         bias=eps128[0:12, :])
                    nc.vector.reciprocal(spair[:, PJ_N:2 * PJ_N], stdd[:])
                    rsb = p1ps1.tile([128, 512], f32, tag="rsb")
                    nc.tensor.matmul(rsb[0:120, 0:PJ_N], bb_sb[:],
                                     spair[:, PJ_N:2 * PJ_N],
                                     start=True, stop=True)
                    t2 = p1w.tile([120, PJ_N], f32, tag="t2")
                    nc.vector.tensor_tensor(t2[:], t1[:], rsb[0:120, 0:PJ_N],
                                            mybir.AluOpType.mult)
                    nc.vector.tensor_scalar(yf[:, s * PJ_N:(s + 1) * PJ_N],
                                            t2[:], vecs["gam_v"][:],
                                            vecs["bet_v"][:],
                                            mybir.AluOpType.mult,
                                            mybir.AluOpType.add)

                # scatter to DRAM [t, (ch, f)]: one DMA per col-group;
                # group j holds t0+24j..t0+24j+23 contiguously across subtiles
                for j in range(4):
                    dst = AP(tensor=qkv2d.tensor,
                             offset=(t0 + 24 * j) * 24 * F,
                             ap=[[F, 24], [24 * F, 4 * PJ_T], [1, F]])
                    nc.sync.dma_start(dst, yf[32 * j:32 * j + 24, :])

        # ================= phase 2: load K/Q emb (transpose) + V ==========
        if "2" not in phases:
            return
        with tc.tile_pool(name="attp", bufs=1) as attp:
            k_eT = []
            q_eT = []
            for ce, (e0, w) in enumerate(((0, 128), (128, 128), (256, 4))):
                kt = attp.tile([128, TP], bf16, name=f"k_eT{ce}")
                qt_ = attp.tile([128, TP], bf16, name=f"q_eT{ce}")
                for sb in range(NSB):
                    nc.sync.dma_start_transpose(
                        kt[0:w, sb * 512:(sb + 1) * 512],
                        qkv2d[sb * 512:(sb + 1) * 512, DF + e0:DF + e0 + w])
                    nc.sync.dma_start_transpose(
                        qt_[0:w, sb * 512:(sb + 1) * 512],
                        qkv2d[sb * 512:(sb + 1) * 512, e0:e0 + w])
                k_eT.append(kt)
                q_eT.append(qt_)

            v_sb = []
            for st in range(NQT):
                vt = attp.tile([128, EF + 1], bf16, name=f"v_sb{st}")
                nc.sync.dma_start(
                    vt[:, 0:EF], qkv2d[st * 128:(st + 1) * 128, 2 * DF:24 * F])
                nc.vector.memset(vt[:, EF:EF + 1], 1.0)
                v_sb.append(vt)

            # ============== phase 3: attention (transposed scores) ========
            if "3" not in phases:
                return
            with tc.tile_pool(name="a3", bufs=2) as a3, \
                 tc.tile_pool(name="a3p", bufs=2) as a3p, \
                 tc.tile_pool(name="a3ps", bufs=2, space="PSUM") as a3ps, \
                 tc.tile_pool(name="a3po", bufs=2, space="PSUM") as a3po:
                for blk in range(NSB):
                    q0 = blk * 512
                    # scores S^T [s-tile, tq(512)] -> exp -> P^T tiles
                    pT = []
                    for st in range(NQT):
                        s_ps = a3ps.tile([128, 512], f32, tag="s_ps")
                        for ce, w in ((0, 128), (1, 128), (2, 4)):
                            nc.tensor.matmul(
                                s_ps[:],
                                k_eT[ce][0:w, st * 128:(st + 1) * 128],
                                q_eT[ce][0:w, q0:q0 + 512],
                                start=(ce == 0), stop=(ce == 2))
                        pb = a3p.tile([128, 512], bf16, tag=f"pb{st}")
                        if st == NQT - 1:
                            nc.vector.memset(pb[:], 0.0)
                            nc.scalar.activation(
                                pb[0:T_PAD0, :], s_ps[0:T_PAD0, :],
                                mybir.ActivationFunctionType.Exp, scale=SCALE)
                        else:
                            nc.scalar.activation(
                                pb[:], s_ps[:],
                                mybir.ActivationFunctionType.Exp, scale=SCALE)
                        pT.append(pb)

                    # O[tq128, ef+1] accumulated over s; col EF = denominator
                    for tq in range(4):
                        o_ps = a3po.tile([128, 1536], f32, tag="o_ps")
                        for st in range(NQT):
                            lw = pT[st][:, tq * 128:(tq + 1) * 128]
                            first, last = (st == 0), (st == NQT - 1)
                            nc.tensor.matmul(o_ps[:, 0:512], lw,
                                             v_sb[st][:, 0:512],
                                             start=first, stop=last)
                            nc.tensor.matmul(o_ps[:, 512:1024], lw,
                                             v_sb[st][:, 512:1024],
                                             start=first, stop=last)
                            nc.tensor.matmul(o_ps[:, 1024:1024 + EF + 1 - 1024],
                                             lw, v_sb[st][:, 1024:EF + 1],
                                             start=first, stop=last)
                        rcp = a3.tile([128, 1], f32, tag="rcp")
                        nc.vector.reciprocal(rcp[:], o_ps[:, EF:EF + 1])
                        o_sb = a3.tile([128, EF], bf16, tag="o_sb")
                        nc.vector.tensor_scalar(o_sb[:], o_ps[:, 0:EF],
                                                rcp[:], None,
                                                mybir.AluOpType.mult)
                        qt = blk * 4 + tq
                        sh, tl0 = qt // 3, (qt % 3) * 128
                        dst = AP(tensor=oint.tensor,
                                 offset=sh * 16 * SHF + tl0 * F,
                                 ap=[[F, 128], [SHF, E], [1, F]])
                        nc.sync.dma_start(dst, o_sb[:])

        # ================= phase 4: AllToAll =================
        if "A" not in phases:
            return
        nc.gpsimd.collective_compute(
            "AllToAll", mybir.AluOpType.bypass,
            replica_groups=[[0, 1, 2, 3, 4, 5, 6, 7]],
            ins=[oint[:]],
            outs=[oall.rearrange("(a c) n -> a (c n)", a=8)],
        )

        # ================= phase 5: final proj + LN + residual ============
        if "5" not in phases:
            return
        x_res = t["x_res"]
        y_out = t["y_out"]
        C2 = 2 * C
        with tc.tile_pool(name="p5", bufs=2) as p5, \
             tc.tile_pool(name="p5ps", bufs=1, space="PSUM") as p5ps:
            nchunks = (SHF + 1023) // 1024
            for k in range(nchunks):
                n0 = k * 1024
                n = min(1024, SHF - n0)
                nsp = [(0, min(512, n))]
                if n > 512:
                    nsp.append((512, n - 512))
                o_c = p5.tile([C2, 1024], bf16, tag="o_c")
                nc.scalar.dma_start(o_c[:, 0:n], oall[:, n0:n0 + n])
                x_c = p5.tile([C2, 1024], f32, tag="x_c")
                nc.sync.dma_start(x_c[:, 0:n], x_res[:, n0:n0 + n])

                y1 = p5ps.tile([C2, 1024], f32, tag="y1")
                for o0, nn in nsp:
                    nc.tensor.matmul(y1[:, o0:o0 + nn], wp_sb[:],
                                     o_c[:, o0:o0 + nn],
                                     start=True, stop=True)
                r1 = p5.tile([C2, 1024], f32, tag="fr1")
                nc.scalar.activation(r1[:, 0:n], y1[:, 0:n],
                                     mybir.ActivationFunctionType.Relu,
                                     bias=fvecs["bp_v"][:], scale=1.0)
                r2 = p5.tile([C2, 1024], f32, tag="fr2")
                nc.scalar.activation(r2[:, 0:n], y1[:, 0:n],
                                     mybir.ActivationFunctionType.Relu,
                                     bias=fvecs["nbp_v"][:], scale=-1.0)
                r2a = p5.tile([C2, 1024], f32, tag="fr2a")
                nc.vector.tensor_scalar(r2a[:, 0:n], r2[:, 0:n],
                                        fvecs["nap_v"][:], None,
                                        mybir.AluOpType.mult)
                s_sb = p5.tile([C2, 1024], bf16, tag="fs")
                nc.gpsimd.tensor_tensor(s_sb[:, 0:n], r1[:, 0:n],
                                        r2a[:, 0:n], mybir.AluOpType.add)

                mu = p5ps.tile([C2, 1024], f32, tag="fmu")
                for o0, nn in nsp:
                    nc.tensor.matmul(mu[:, o0:o0 + nn], ones_sb[:],
                                     s_sb[:, o0:o0 + nn],
                                     start=True, stop=True)
                t1 = p5.tile([C2, 1024], f32, tag="ft1")
                nc.vector.tensor_tensor(t1[:, 0:n], s_sb[:, 0:n],
                                        mu[:, 0:n], mybir.AluOpType.subtract)
                sq = p5.tile([C2, 1024], bf16, tag="fsq")
                nc.scalar.activation(sq[:, 0:n], t1[:, 0:n],
                                     mybir.ActivationFunctionType.Square)
                vv = p5ps.tile([C2, 1024], f32, tag="fvar")
                for o0, nn in nsp:
                    nc.tensor.matmul(vv[:, o0:o0 + nn], ones_sb[:],
                                     sq[:, o0:o0 + nn],
                                     start=True, stop=True)
                stdd = p5.tile([C2, 1024], f32, tag="fstd")
                nc.scalar.activation(stdd[:, 0:n], vv[:, 0:n],
                                     mybir.ActivationFunctionType.Sqrt,
                                     bias=eps128[:])
                rstd = p5.tile([C2, 1024], f32, tag="frstd")
                nc.vector.reciprocal(rstd[:, 0:n], stdd[:, 0:n])
                yn = p5.tile([C2, 1024], f32, tag="fyn")
                nc.gpsimd.tensor_tensor(yn[:, 0:n], t1[:, 0:n],
                                        rstd[:, 0:n], mybir.AluOpType.mult)
                yg = p5.tile([C2, 1024], f32, tag="fyg")
                nc.vector.tensor_scalar(yg[:, 0:n], yn[:, 0:n],
                                        fvecs["gp_v"][:], fvecs["betp_v"][:],
                                        mybir.AluOpType.mult,
                                        mybir.AluOpType.add)
                yo = p5.tile([C2, 1024], f32, tag="fyo")
                nc.gpsimd.tensor_tensor(yo[:, 0:n], yg[:, 0:n],
                                        x_c[:, 0:n], mybir.AluOpType.add)
                nc.sync.dma_start(y_out[:, n0:n0 + n], yo[:, 0:n])


_PROGRAM = None


def _get_program():
    global _PROGRAM
    if _PROGRAM is None:
        _PROGRAM = _build_program()
    return _PROGRAM


def _core_inputs(inp, c):
    b, h = c // 4, c % 4
    x = np.asarray(inp["x"], np.float32)
    xb = np.zeros((B, C, TP, F), np.float32)
    xb[:, :, :T, :] = x
    x_pad = np.ascontiguousarray(xb[b].reshape(C, TFP))
    # final-stage residual: eighth-shard c of BOTH batches, stacked [2C, SHF]
    xs = xb[:, :, SH * c:SH * (c + 1), :].reshape(B * C, SHF)
    x_res = np.ascontiguousarray(xs)

    Wq, Wk, Wv = (np.asarray(inp[k], np.float32) for k in ("Wq", "Wk", "Wv"))
    bq, bk, bv = (np.asarray(inp[k], np.float32) for k in ("bq", "bk", "bv"))
    aq, ak, av = (np.asarray(inp[k], np.float32) for k in ("aq", "ak", "av"))
    gq, gk, gv = (np.asarray(inp[k], np.float32) for k in ("gq", "gk", "gv"))
    btq, btk, btv = (np.asarray(inp[k], np.float32)
                     for k in ("betaq", "betak", "betav"))

    w24 = np.concatenate([Wq[h], Wk[h], Wv[h]], axis=0)     # [24, C]
    b24 = np.concatenate([bq[h], bk[h], bv[h]])             # [24]
    a24 = np.concatenate([np.full(D, aq[h]), np.full(D, ak[h]),
                          np.full(E, av[h])]).astype(np.float32)
    g24 = np.concatenate([gq[h], gk[h], gv[h]])
    bt24 = np.concatenate([btq[h], btk[h], btv[h]])

    w4 = np.zeros((C, 128), np.float32)
    bias_v = np.zeros((120, 1), np.float32)
    na_v = np.zeros((120, 1), np.float32)
    gam_v = np.zeros((120, 1), np.float32)
    bet_v = np.zeros((120, 1), np.float32)
    G = np.zeros((120, 12), np.float32)
    Bbm = np.zeros((12, 120), np.float32)
    for j in range(4):
        r = 32 * j
        w4[:, r:r + 24] = w24.T
        bias_v[r:r + 24, 0] = b24
        na_v[r:r + 24, 0] = -a24
        gam_v[r:r + 24, 0] = g24
        bet_v[r:r + 24, 0] = bt24
        G[r:r + 4, 3 * j + 0] = 0.25
        G[r + 4:r + 8, 3 * j + 1] = 0.25
        G[r + 8:r + 24, 3 * j + 2] = 1.0 / 16.0
        Bbm[3 * j + 0, r:r + 4] = 1.0
        Bbm[3 * j + 1, r + 4:r + 8] = 1.0
        Bbm[3 * j + 2, r + 8:r + 24] = 1.0

    Wp = np.asarray(inp["Wp"], np.float32)
    wp2 = np.zeros((2 * C, 2 * C), np.float32)
    wp2[:C, :C] = Wp.T
    wp2[C:, C:] = Wp.T
    ones2 = np.zeros((2 * C, 2 * C), np.float32)
    ones2[:C, :C] = 1.0 / 64.0
    ones2[C:, C:] = 1.0 / 64.0
    bp = np.asarray(inp["bp"], np.float32)
    ap = np.float32(inp["ap"])
    gp = np.asarray(inp["gp"], np.float32)
    betp = np.asarray(inp["betap"], np.float32)

    return {
        "x_pad": x_pad.astype(BF16),
        "x_res": x_res,
        "w4": w4.astype(BF16),
        "bias_v": bias_v,
        "nbias_v": -bias_v,
        "na_v": na_v,
        "gam_v": gam_v,
        "bet_v": bet_v,
        "Gm": G.astype(BF16),
        "Bb": Bbm.astype(BF16),
        "wp2": wp2.astype(BF16),
        "ones128": ones2.astype(BF16),
        "nap_v": np.full((2 * C, 1), -ap, np.float32),
        "bp_v": np.tile(bp, 2).reshape(2 * C, 1),
        "nbp_v": np.tile(-bp, 2).reshape(2 * C, 1),
        "gp_v": np.tile(gp, 2).reshape(2 * C, 1),
        "betp_v": np.tile(betp, 2).reshape(2 * C, 1),
    }


def gather_output(results):
    y = np.empty((B, C, T, F), np.float32)
    for c in range(8):
        sh = np.asarray(results[c]["y_shard"], np.float32).reshape(B, C, SH, F)
        t0, t1 = SH * c, min(SH * (c + 1), T)
        if t1 > t0:
            y[:, :, t0:t1, :] = sh[:, :, :t1 - t0, :]
    return y


def kernel(**inputs):
    nc = _get_program()
    in_maps = [_core_inputs(inputs, c) for c in range(8)]
    res = run_bass_kernel_spmd(nc, in_maps, core_ids=list(range(8)))
    return gather_output(res.results)
